# revision 10
# baseline (speedup 1.0000x reference)
"""ButterflyLinear TRN2 kernel — 8-core data-parallel dense matmul (v4).

The module  out = blockdiag(shuffle(blockdiag(x, stage1)) @ mix_w.T, stage2)
is a fixed linear map on the 2048-d feature axis: fold it into one dense
A [2048, 2048] on the host and run y = x @ A data-parallel (2048 tokens/core).

Device schedule (per core): x-stationary / A-moving.
  for t in 16 token-tiles:             # stationary x[t,k]: [128 k, 128 tok]
    for k in 16 contraction tiles:     #   one weight load per (t,k), reused
      for o in 4 output slabs of 512:  #   moving A[k,o]: [128 k, 512 out]
        psum[o] (+)= x[t,k].T @ A[k,o]
    drain 4 psum banks -> one fused [128, 2048] f32 HWDGE store (token-major)

Ring assignment (measured: SWDGE/gpsimd stores are starved by DVE perf-mode
copies and stall the pipeline ~1us per store):
  sync (SP):     x tile loads          scalar (Act): A chunk loads
  vector (DVE):  psum copies + fused y stores (HWDGE)   gpsimd: unused
Operands fp16 (half DMA bytes, 1 cycle/row PE), fp32 PSUM accumulate, fp32 out.
"""

import sys

if "/opt/trn_rl_repo" not in sys.path:
    sys.path.insert(0, "/opt/trn_rl_repo")

import numpy as np

IN_F = 2048
OUT_F = 2048
BS = 64
NIB = IN_F // BS
NOB = OUT_F // BS
N_CORES = 8
TOK_PC = 2048  # tokens per core (16384 / 8)

P = 128
KT = IN_F // P   # 16 contraction tiles
NT = 512         # moving (out-feature) tile
NO = OUT_F // NT # 4 out slabs
TT = TOK_PC // P # 16 token tiles per core
BODY_REPS = 2  # kernel executions per For_i iteration (timing builds)

_CACHE = {}


def _build(loop_iters: int = 0):
    """Build + compile the per-core Bass program (SPMD, same on all cores).

    loop_iters > 0 wraps the body in a hardware For_i loop (timing builds)."""
    import contextlib

    import concourse.mybir as mybir
    import concourse.tile as tile
    from concourse import bacc

    nc = bacc.Bacc(None, target_bir_lowering=False, debug=False)
    f32 = mybir.dt.float32
    f16 = mybir.dt.float16

    a_ext = nc.declare_dram_parameter("a", [128, KT, OUT_F], f16, isOutput=False)
    x_ext = nc.declare_dram_parameter("x", [TT, 128, KT, P], f16, isOutput=False)
    y_ext = nc.declare_dram_parameter("y", [TOK_PC, OUT_F], f32, isOutput=True)

    with tile.TileContext(nc) as tc:
        with (
            tc.tile_pool(name="a_pool", bufs=1) as a_pool,
            tc.tile_pool(name="x_pool", bufs=3) as x_pool,
            tc.tile_pool(name="o_pool", bufs=3) as o_pool,
            tc.tile_pool(name="ps", bufs=2, space="PSUM") as ps_pool,
        ):
            loop_cm = (
                tc.For_i(0, loop_iters, 1, hint_engines=(mybir.EngineType.PE,))
                if loop_iters
                else contextlib.nullcontext()
            )
            with loop_cm:
             # Timing builds run two full kernel executions per For_i
             # iteration: halves the loop-boundary sync frequency and smooths
             # the timing slope (test.py divides by BODY_REPS).  The one-shot
             # build (loop_iters=0) runs the body once.
             for _rep in range(BODY_REPS if loop_iters else 1):
                # A resident, loaded in per-k chunks so compute can start as
                # soon as chunk k=0 lands.
                a_t = []
                for k in range(KT):
                    ak = a_pool.tile([P, OUT_F], f16, tag=f"a{k}")
                    nc.scalar.dma_start(ak[:], a_ext[:, k, :])
                    a_t.append(ak)
                def load_x(t):
                    xt = x_pool.tile([P, KT, P], f16, tag="x")
                    nc.sync.dma_start(xt[:], x_ext[t])
                    return xt

                # manual 2-ahead software pipeline of x tiles: issue the load
                # for t+2 during group t so store-waits on the same in-order
                # SP queue never delay the next group's stationary operand
                xts = [load_x(0), load_x(1)]
                for t in range(TT):
                    if t + 2 < TT:
                        xts.append(load_x(t + 2))
                    xt = xts[t]
                    # one 4-bank psum tile per token group: each matmul writes
                    # one bank-slice (start=True clears only that bank), and a
                    # single drain copy covers all 4 banks
                    psp = ps_pool.tile([P, NO, NT], f32, name="psp")
                    for k in range(KT):
                        for o in range(NO):
                            nc.tensor.matmul(
                                psp[:, o, :],
                                xt[:, k, :],
                                a_t[k][:, o * NT : (o + 1) * NT],
                                start=(k == 0),
                                stop=(k == KT - 1),
                            )
                    # fused per-t store: one HWDGE DMA of [128, 2048] f32
                    # (8KB contiguous per partition) on the SP ring; gpsimd
                    # SWDGE is starved by DVE perf-mode copies, so avoid it
                    o_t = o_pool.tile([P, NO, NT], f32, tag="o")
                    nc.vector.tensor_copy(o_t[:], psp[:])
                    nc.sync.dma_start(
                        y_ext[t * P : (t + 1) * P, :],
                        o_t[:].rearrange("p a b -> p (a b)"),
                    )
    nc.compile()
    return nc


def get_nc(repeats: int = 1, loop_iters: int = 0):
    key = ("nc", loop_iters)
    if key not in _CACHE:
        _CACHE[key] = _build(loop_iters)
    return _CACHE[key]


def compose_A(stage1: np.ndarray, stage2: np.ndarray, mix_w: np.ndarray) -> np.ndarray:
    """Fold stage1 ∘ shuffle ∘ mix ∘ stage2 into one dense [IN_F, OUT_F] map."""
    j = np.arange(IN_F)
    sig = (j % BS) * NIB + j // BS  # shuffle: h2[:, j] = h1[:, sig(j)]
    Ms = np.empty((IN_F, OUT_F), dtype=np.float64)
    Ms[sig, :] = mix_w.T.astype(np.float64)  # y = h1 @ Ms
    A_mid = np.einsum(
        "gcd,gdo->gco",
        stage1.reshape(NIB, BS, BS).astype(np.float64),
        Ms.reshape(NIB, BS, OUT_F),
    ).reshape(IN_F, OUT_F)
    A = np.einsum(
        "igc,gcd->igd",
        A_mid.reshape(IN_F, NOB, BS),
        stage2.reshape(NOB, BS, BS).astype(np.float64),
    ).reshape(IN_F, OUT_F)
    return A.astype(np.float32)


def make_in_maps(x, stage1, stage2, mix_w):
    A = compose_A(np.asarray(stage1), np.asarray(stage2), np.asarray(mix_w))
    # [p, k, o] = A[k*128+p, o]
    A_dev = np.ascontiguousarray(
        A.reshape(KT, P, OUT_F).transpose(1, 0, 2).astype(np.float16)
    )
    x_flat = np.ascontiguousarray(np.asarray(x), dtype=np.float32).reshape(-1, IN_F)
    in_maps = []
    for c in range(N_CORES):
        shard = x_flat[c * TOK_PC : (c + 1) * TOK_PC, :]
        # [t, p, k, j] = shard[t*128+j, k*128+p]
        x4 = np.ascontiguousarray(
            shard.reshape(TT, P, KT, P).transpose(0, 3, 2, 1).astype(np.float16)
        )
        in_maps.append({"a": A_dev, "x": x4})
    return in_maps


def assemble_output(results, batch_shape):
    y_flat = np.empty((N_CORES * TOK_PC, OUT_F), dtype=np.float32)
    for c in range(N_CORES):
        y_flat[c * TOK_PC : (c + 1) * TOK_PC, :] = results[c]["y"]
    return y_flat.reshape(*batch_shape, OUT_F)


def kernel(x, stage1, stage2, mix_w):
    from concourse.bass_utils import run_bass_kernel_spmd

    batch_shape = np.asarray(x).shape[:-1]
    nc = get_nc()
    in_maps = make_in_maps(x, stage1, stage2, mix_w)
    res = run_bass_kernel_spmd(nc, in_maps, list(range(N_CORES)))
    return assemble_output(res.results, batch_shape)
